# revision 17
# baseline (speedup 1.0000x reference)
"""AffineCoupling Trainium2 kernel (8 NeuronCores, data-parallel over batch).

Reference computation (per sample s):
  w = emb@Wa.T+ba -> [512,3] dyn depthwise weights; b = emb@Wb.T+bb -> [512]
  x1 = relu(depthwise3(h1_bcast, w) + b)        [512, 2048]
  x2 = relu(W1 @ x1 + b1)                       [512, 2048]
  x3 = conv3(x2, W2) + b2                       [32, 2048]
  s_ = sigmoid(x3[:16]+2)+1e-7 ; m = x3[16:]
  h2' = s_*(h2+m); logdet = sum log s_
Host computes w,b (25 MFLOP) and packs weights; device does everything else.

Device mapping per core (8 samples):
 - L=2048 split into 5 column windows (4x 512-wide compute -> 510 out, +8).
 - depthwise: one K=48 matmul per 128-channel tile; rhs = 3 tap-shifted
   copies of the 16 h1 rows stacked on partitions (3 DMAs/sample), lhsT
   zero-padded so each tile selects its 4 h1 channels per tap.
 - W1 1x1 conv: 16 [128x128x512] matmuls/window, f32r full-rate fp32.
 - W2 3-tap conv: taps in M (M=96), 4 matmuls/window; tap-shift folded by
   DVE adds after partition realignment via SBUF-SBUF DMA.
 - sigmoid per window; Ln batched at the end (one ACT table load each).
"""

import os
import sys

sys.path.insert(0, "/opt/trn_rl_repo")

import ml_dtypes
import numpy as np

import concourse.mybir as mybir
import concourse.tile as tile
from concourse import bacc

KW = 3
NCORES = 8
S = 8          # samples per core
NSQC = 16      # h1 channels
L = 2048
PAD = 2        # left zero pad cols in padded h1 row buffer
F32 = mybir.dt.float32
MM_DT = mybir.dt.float32r  # fp32 data, full-rate PE path
USE_BF16 = os.environ.get("AC_BF16", "1") == "1"
# W1/W2 stage dtype: bf16 halves LDWEIGHTS (FWL) and keeps PE warm; the
# depthwise stays f32r so raw h1 is never rounded.
W_DT = mybir.dt.bfloat16 if USE_BF16 else MM_DT

# (win_start, win_w, out_start, out_w): compute window = out window +1 halo col
# each side; psum bank limit 512 -> 510 net output cols per window.
WINDOWS = [(2039, 10, 2040, 8)] + [(510 * i - 1, 512, 510 * i, 510) for i in range(4)]


def build_bass():
    nc = bacc.Bacc()
    AF = mybir.ActivationFunctionType
    OP = mybir.AluOpType

    h1 = nc.dram_tensor("h1", [S, 48, L + 2], W_DT, kind="ExternalInput")
    h2 = nc.dram_tensor("h2", [S, NSQC, L], F32, kind="ExternalInput")
    dw = nc.dram_tensor("dw", [S, 48, 4, 128], W_DT, kind="ExternalInput")
    bsb = nc.dram_tensor("bsb", [128, S, 4], F32, kind="ExternalInput")
    w1t = nc.dram_tensor("w1t", [512, 512], W_DT, kind="ExternalInput")
    w2t = nc.dram_tensor("w2t", [512, 96], W_DT, kind="ExternalInput")
    b1p = nc.dram_tensor("b1p", [128, 4], F32, kind="ExternalInput")
    b2s = nc.dram_tensor("b2s", [16, 1], F32, kind="ExternalInput")
    b2m = nc.dram_tensor("b2m", [16, 1], F32, kind="ExternalInput")
    h2o = nc.dram_tensor("h2o", [S, NSQC, L], F32, kind="ExternalOutput")
    ld = nc.dram_tensor("ld", [1, S], F32, kind="ExternalOutput")

    with tile.TileContext(nc) as tc:
        with (
            tc.tile_pool(name="consts", bufs=1) as consts,
            tc.tile_pool(name="persample", bufs=2) as persample,
            tc.tile_pool(name="work", bufs=2) as work,
            tc.tile_pool(name="io", bufs=3) as io,
            tc.tile_pool(name="pdw", bufs=1, space="PSUM") as pdw,
            tc.tile_pool(name="px2", bufs=3, space="PSUM") as px2,
            tc.tile_pool(name="py", bufs=1, space="PSUM") as py,
        ):
            # ---- constants ----
            w1_sb = consts.tile([128, 4, 512], W_DT)
            for kt in range(4):
                nc.sync.dma_start(w1_sb[:, kt, :], w1t[128 * kt : 128 * (kt + 1), :])
            w2_sb = consts.tile([128, 4, 96], W_DT)
            for kt in range(4):
                nc.sync.dma_start(w2_sb[:, kt, :], w2t[128 * kt : 128 * (kt + 1), :])
            b_sb = consts.tile([128, S, 4], F32)
            nc.sync.dma_start(b_sb[:], bsb[:])
            b1_sb = consts.tile([128, 4], F32)
            nc.sync.dma_start(b1_sb[:], b1p[:])
            b2s_sb = consts.tile([16, 1], F32)
            nc.sync.dma_start(b2s_sb[:], b2s[:])
            b2m_sb = consts.tile([16, 1], F32)
            nc.sync.dma_start(b2m_sb[:], b2m[:])
            ones_sb = consts.tile([16, 1], F32)
            nc.vector.memset(ones_sb[:], 1.0)
            s_all = consts.tile([16, S, L], W_DT)
            ld_all = consts.tile([16, S], F32)

            def sample_dmas(s):
                stk_sb = persample.tile(
                    [48, L + 2], W_DT, tag="stk", name="stk"
                )
                nc.sync.dma_start(stk_sb[:], h1[s])
                dw_sb = persample.tile([48, 4, 128], W_DT, tag="dw", name="dwsb")
                nc.sync.dma_start(dw_sb[:], dw[s])
                return stk_sb, dw_sb

            def stage1(s, stk_sb, dw_sb, widx):
                ws, ww, os_, ow = WINDOWS[widx]
                pts = [
                    pdw.tile([128, 512], F32, tag=f"dw{t}", name=f"pdw{t}")
                    for t in range(4)
                ]
                for t in range(4):
                    nc.tensor.matmul(
                        pts[t][:, :ww],
                        dw_sb[:, t, :],
                        stk_sb[:, ws + 1 : ws + 1 + ww],
                        start=True,
                        stop=True,
                    )
                x1_sb = work.tile([128, 4, 512], W_DT, tag="x1", name="x1sb")
                for t in range(4):
                    nc.vector.tensor_scalar(
                        out=x1_sb[:, t, :ww],
                        in0=pts[t][:, :ww],
                        scalar1=b_sb[:, s, t : t + 1],
                        scalar2=0.0,
                        op0=OP.add,
                        op1=OP.max,
                    )
                return x1_sb

            jobs = [(s, w) for s in range(S) for w in range(len(WINDOWS))]
            stk_sb, dw_sb = sample_dmas(0)
            x1_cur = stage1(0, stk_sb, dw_sb, 0)
            for i, (s, widx) in enumerate(jobs):
                ws, ww, os_, ow = WINDOWS[widx]
                pyt = py.tile([128, 512], F32, tag="py")
                h2_sb = io.tile([16, 512], F32, tag="h2in")
                nc.sync.dma_start(h2_sb[:, :ow], h2[s, :, os_ : os_ + ow])

                # --- stage 2: 1x1 conv W1 (x2 = relu(W1@x1 + b1))
                x2_sb = work.tile([128, 4, 512], W_DT, tag="x2")
                for ot in range(4):
                    ps = px2.tile([128, 512], F32, tag="px2")
                    for kt in range(4):
                        nc.tensor.matmul(
                            ps[:, :ww],
                            w1_sb[:, kt, 128 * ot : 128 * (ot + 1)],
                            x1_cur[:, kt, :ww],
                            start=(kt == 0),
                            stop=(kt == 3),
                        )
                    nc.scalar.activation(
                        out=x2_sb[:, ot, :ww],
                        in_=ps[:, :ww],
                        func=AF.Relu,
                        bias=b1_sb[:, ot : ot + 1],
                        scale=1.0,
                    )

                # prefetch next window's stage 1 while ACT drains x2 relus
                if i + 1 < len(jobs):
                    s2, w2 = jobs[i + 1]
                    if w2 == 0:
                        stk_sb, dw_sb = sample_dmas(s2)
                    x1_cur = stage1(s2, stk_sb, dw_sb, w2)

                # --- stage 3: 3-tap conv W2 as M=96 matmul (taps in M)
                for kt in range(4):
                    nc.tensor.matmul(
                        pyt[:96, :ww],
                        w2_sb[:, kt, :],
                        x2_sb[:, kt, :ww],
                        start=(kt == 0),
                        stop=(kt == 3),
                    )
                y_sb = work.tile([96, 512], W_DT, tag="y")
                nc.scalar.copy(out=y_sb[:, :ww], in_=pyt[:96, :ww])
                # reference zero-pads x2 outside [0, L): mask tap cols that
                # read the out-of-range x2 values
                if ws == -1:
                    nc.vector.memset(y_sb[0:32, 0:1], 0.0)
                # realign tap blocks to partitions 0:32 (engines cannot
                # cross partitions; DMA can)
                ya_sb = work.tile([32, 512], W_DT, tag="ya")
                yb_sb = work.tile([32, 512], W_DT, tag="yb")
                nc.sync.dma_start(ya_sb[:, :ww], y_sb[32:64, :ww])
                nc.sync.dma_start(yb_sb[:, :ww], y_sb[64:96, :ww])
                if ws + ww > L:
                    nc.vector.memset(yb_sb[:, L - ws : L - ws + 1], 0.0)
                # x3[j, n] = y0[j, n] + y1[j, n+1] + y2[j, n+2]
                x3_sb = work.tile([32, 512], W_DT, tag="x3")
                nc.vector.tensor_add(
                    out=x3_sb[:, :ow],
                    in0=y_sb[0:32, 0:ow],
                    in1=ya_sb[:, 1 : 1 + ow],
                )
                nc.vector.tensor_add(
                    out=x3_sb[:, :ow],
                    in0=x3_sb[:, :ow],
                    in1=yb_sb[:, 2 : 2 + ow],
                )
                # m half joins h2/s on partitions 0:16 -> one more remap
                x3m_sb = work.tile([16, 512], W_DT, tag="x3m")
                nc.sync.dma_start(x3m_sb[:, :ow], x3_sb[16:32, :ow])

                # s_ = sigmoid(x3_s + (b2[:16]+2))  (skip +1e-7: err <=1e-7)
                nc.scalar.activation(
                    out=s_all[:, s, os_ : os_ + ow],
                    in_=x3_sb[0:16, :ow],
                    func=AF.Sigmoid,
                    bias=b2s_sb,
                    scale=1.0,
                )
                # u = (x3_m + b2m) + h2 ; h2' = u * s_
                u_sb = io.tile([16, 512], F32, tag="u")
                nc.vector.scalar_tensor_tensor(
                    out=u_sb[:, :ow],
                    in0=x3m_sb[:, :ow],
                    scalar=b2m_sb,
                    in1=h2_sb[:, :ow],
                    op0=OP.add,
                    op1=OP.add,
                )
                ho_sb = io.tile([16, 512], F32, tag="ho")
                nc.vector.tensor_mul(
                    out=ho_sb[:, :ow],
                    in0=u_sb[:, :ow],
                    in1=s_all[:, s, os_ : os_ + ow],
                )
                nc.sync.dma_start(h2o[s, :, os_ : os_ + ow], ho_sb[:, :ow])

            # ---- phase 2: logdet = sum log s_  (one Ln table load) ----
            for s in range(S):
                lns = work.tile([16, L], F32, tag="lns")
                nc.scalar.activation(
                    out=lns[:],
                    in_=s_all[:, s, :],
                    func=AF.Ln,
                    accum_out=ld_all[:, s : s + 1],
                )
            pld = py.tile([128, 512], F32, tag="py")
            nc.tensor.matmul(
                pld[0:1, 0:S], ones_sb[:], ld_all[:], start=True, stop=True
            )
            ldo = io.tile([1, S], F32, tag="ldo")
            nc.vector.tensor_copy(ldo[:], pld[0:1, 0:S])
            nc.sync.dma_start(ld[:], ldo[:])

    nc.compile()
    return nc


_NC = None


def _get_nc():
    global _NC
    if _NC is None:
        _NC = build_bass()
    return _NC


def _pack_host(h, emb, Wa, ba, Wb, bb, W1, b1, W2, b2):
    f = np.float32
    emb = np.asarray(emb, f)
    w = (emb @ np.asarray(Wa, f).T + np.asarray(ba, f)).reshape(64, 512, KW)
    bvec = emb @ np.asarray(Wb, f).T + np.asarray(bb, f)  # [64, 512]
    wdt = ml_dtypes.bfloat16 if USE_BF16 else f
    wr = w.reshape(64, 4, 4, 32, KW)  # s, t, q, r, k
    # K=48 stacked layout: row 16k + 4t + q selects h1 chan 4t+q, tap k
    dwp_f = np.zeros((64, 48, 4, 128), f)
    for k in range(KW):
        for t in range(4):
            for q in range(4):
                dwp_f[:, 16 * k + 4 * t + q, t, 32 * q : 32 * q + 32] = wr[:, t, q, :, k]
    dwp = dwp_f.astype(wdt)
    bsb = np.ascontiguousarray(bvec.reshape(64, 4, 128).transpose(2, 0, 1))
    w1t = np.ascontiguousarray(np.asarray(W1, f)[:, :, 0].T.astype(wdt))
    w2t = np.ascontiguousarray(
        np.asarray(W2, f).transpose(1, 2, 0).reshape(512, 96).astype(wdt)
    )
    b1p = np.ascontiguousarray(np.asarray(b1, f).reshape(4, 128).T)
    b2 = np.asarray(b2, f)
    b2sp = np.ascontiguousarray((b2[:16] + 2.0).reshape(16, 1))
    b2mp = np.ascontiguousarray(b2[16:].reshape(16, 1))
    h = np.asarray(h, f)
    h1f = np.ascontiguousarray(h[:, :16, :])
    h2f = np.ascontiguousarray(h[:, 16:, :])
    h1pad = np.zeros((64, 16, PAD + L + 2), wdt)
    h1pad[:, :, PAD : PAD + L] = h1f.astype(wdt)
    hstk = np.zeros((64, 48, L + 2), wdt)
    for k in range(KW):
        hstk[:, 16 * k : 16 * (k + 1), :] = h1pad[:, :, k : k + L + 2]
    in_maps = []
    for c in range(NCORES):
        sl = slice(S * c, S * (c + 1))
        in_maps.append(
            {
                "h1": np.ascontiguousarray(hstk[sl]),
                "h2": np.ascontiguousarray(h2f[sl]),
                "dw": np.ascontiguousarray(dwp[sl]),
                "bsb": np.ascontiguousarray(bsb[:, sl, :]),
                "w1t": w1t,
                "w2t": w2t,
                "b1p": b1p,
                "b2s": b2sp,
                "b2m": b2mp,
            }
        )
    return h1f, in_maps


def run_on_device(in_maps, **kwargs):
    from concourse import bass_utils

    return bass_utils.run_bass_kernel_spmd(
        _get_nc(), in_maps, core_ids=list(range(NCORES)), **kwargs
    )


def kernel(h, emb, Wa, ba, Wb, bb, W1, b1, W2, b2):
    h1f, in_maps = _pack_host(h, emb, Wa, ba, Wb, bb, W1, b1, W2, b2)
    res = run_on_device(in_maps)
    outs = res.results
    h2o = np.concatenate([o["h2o"] for o in outs], axis=0)
    ldv = np.concatenate([o["ld"][0] for o in outs], axis=0)
    h_out = np.concatenate([h1f, h2o], axis=1)
    return h_out, ldv


# revision 18
# speedup vs baseline: 1.0054x; 1.0054x over previous
"""AffineCoupling Trainium2 kernel (8 NeuronCores, data-parallel over batch).

Reference computation (per sample s):
  w = emb@Wa.T+ba -> [512,3] dyn depthwise weights; b = emb@Wb.T+bb -> [512]
  x1 = relu(depthwise3(h1_bcast, w) + b)        [512, 2048]
  x2 = relu(W1 @ x1 + b1)                       [512, 2048]
  x3 = conv3(x2, W2) + b2                       [32, 2048]
  s_ = sigmoid(x3[:16]+2)+1e-7 ; m = x3[16:]
  h2' = s_*(h2+m); logdet = sum log s_
Host computes w,b (25 MFLOP) and packs weights; device does everything else.

Device mapping per core (8 samples):
 - L=2048 split into 5 column windows (4x 512-wide compute -> 510 out, +8).
 - depthwise: one K=48 matmul per 128-channel tile; rhs = 3 tap-shifted
   copies of the 16 h1 rows stacked on partitions (3 DMAs/sample), lhsT
   zero-padded so each tile selects its 4 h1 channels per tap.
 - W1 1x1 conv: 16 [128x128x512] matmuls/window, f32r full-rate fp32.
 - W2 3-tap conv: taps in M (M=96), 4 matmuls/window; tap-shift folded by
   DVE adds after partition realignment via SBUF-SBUF DMA.
 - sigmoid per window; Ln batched at the end (one ACT table load each).
"""

import os
import sys

sys.path.insert(0, "/opt/trn_rl_repo")

import ml_dtypes
import numpy as np

import concourse.mybir as mybir
import concourse.tile as tile
from concourse import bacc

KW = 3
NCORES = 8
S = 8          # samples per core
NSQC = 16      # h1 channels
L = 2048
PAD = 2        # left zero pad cols in padded h1 row buffer
F32 = mybir.dt.float32
MM_DT = mybir.dt.float32r  # fp32 data, full-rate PE path
USE_BF16 = os.environ.get("AC_BF16", "1") == "1"
# W1/W2 stage dtype: bf16 halves LDWEIGHTS (FWL) and keeps PE warm; the
# depthwise stays f32r so raw h1 is never rounded.
W_DT = mybir.dt.bfloat16 if USE_BF16 else MM_DT

# (win_start, win_w, out_start, out_w): compute window = out window +1 halo col
# each side; psum bank limit 512 -> 510 net output cols per window.
WINDOWS = [(2039, 10, 2040, 8)] + [(510 * i - 1, 512, 510 * i, 510) for i in range(4)]


def build_bass():
    nc = bacc.Bacc()
    AF = mybir.ActivationFunctionType
    OP = mybir.AluOpType

    h1 = nc.dram_tensor("h1", [S, 48, L + 2], W_DT, kind="ExternalInput")
    h2 = nc.dram_tensor("h2", [S, NSQC, L], F32, kind="ExternalInput")
    dw = nc.dram_tensor("dw", [S, 48, 4, 128], W_DT, kind="ExternalInput")
    bsb = nc.dram_tensor("bsb", [128, S, 4], F32, kind="ExternalInput")
    w1t = nc.dram_tensor("w1t", [512, 512], W_DT, kind="ExternalInput")
    w2t = nc.dram_tensor("w2t", [512, 96], W_DT, kind="ExternalInput")
    b1p = nc.dram_tensor("b1p", [128, 4], F32, kind="ExternalInput")
    b2s = nc.dram_tensor("b2s", [16, 1], F32, kind="ExternalInput")
    b2m = nc.dram_tensor("b2m", [16, 1], F32, kind="ExternalInput")
    h2o = nc.dram_tensor("h2o", [S, NSQC, L], F32, kind="ExternalOutput")
    ld = nc.dram_tensor("ld", [1, S], F32, kind="ExternalOutput")

    with tile.TileContext(nc) as tc:
        with (
            tc.tile_pool(name="consts", bufs=1) as consts,
            tc.tile_pool(name="persample", bufs=2) as persample,
            tc.tile_pool(name="work", bufs=2) as work,
            tc.tile_pool(name="io", bufs=3) as io,
            tc.tile_pool(name="pdw", bufs=1, space="PSUM") as pdw,
            tc.tile_pool(name="px2", bufs=3, space="PSUM") as px2,
            tc.tile_pool(name="py", bufs=1, space="PSUM") as py,
        ):
            # ---- constants ----
            w1_sb = consts.tile([128, 4, 512], W_DT)
            for kt in range(4):
                nc.sync.dma_start(w1_sb[:, kt, :], w1t[128 * kt : 128 * (kt + 1), :])
            w2_sb = consts.tile([128, 4, 96], W_DT)
            for kt in range(4):
                nc.sync.dma_start(w2_sb[:, kt, :], w2t[128 * kt : 128 * (kt + 1), :])
            b_sb = consts.tile([128, S, 4], F32)
            nc.sync.dma_start(b_sb[:], bsb[:])
            b1_sb = consts.tile([128, 4], F32)
            nc.sync.dma_start(b1_sb[:], b1p[:])
            b2s_sb = consts.tile([16, 1], F32)
            nc.sync.dma_start(b2s_sb[:], b2s[:])
            b2m_sb = consts.tile([16, 1], F32)
            nc.sync.dma_start(b2m_sb[:], b2m[:])
            ones_sb = consts.tile([16, 1], F32)
            nc.vector.memset(ones_sb[:], 1.0)
            s_all = consts.tile([16, S, L], W_DT)
            ld_all = consts.tile([16, S], F32)

            def sample_dmas(s):
                stk_sb = persample.tile(
                    [48, L + 2], W_DT, tag="stk", name="stk"
                )
                for k in range(KW):
                    nc.sync.dma_start(
                        stk_sb[16 * k : 16 * (k + 1), :], h1[s, 16 * k : 16 * (k + 1), :]
                    )
                dw_sb = persample.tile([48, 4, 128], W_DT, tag="dw", name="dwsb")
                nc.sync.dma_start(dw_sb[:], dw[s])
                return stk_sb, dw_sb

            def stage1(s, stk_sb, dw_sb, widx):
                ws, ww, os_, ow = WINDOWS[widx]
                pts = [
                    pdw.tile([128, 512], F32, tag=f"dw{t}", name=f"pdw{t}")
                    for t in range(4)
                ]
                for t in range(4):
                    nc.tensor.matmul(
                        pts[t][:, :ww],
                        dw_sb[:, t, :],
                        stk_sb[:, ws + 1 : ws + 1 + ww],
                        start=True,
                        stop=True,
                    )
                x1_sb = work.tile([128, 4, 512], W_DT, tag="x1", name="x1sb")
                for t in range(4):
                    nc.vector.tensor_scalar(
                        out=x1_sb[:, t, :ww],
                        in0=pts[t][:, :ww],
                        scalar1=b_sb[:, s, t : t + 1],
                        scalar2=0.0,
                        op0=OP.add,
                        op1=OP.max,
                    )
                return x1_sb

            jobs = [(s, w) for s in range(S) for w in range(len(WINDOWS))]
            cur_dmas = sample_dmas(0)
            nxt_dmas = sample_dmas(1)
            stk_sb, dw_sb = cur_dmas
            x1_cur = stage1(0, stk_sb, dw_sb, 0)
            for i, (s, widx) in enumerate(jobs):
                ws, ww, os_, ow = WINDOWS[widx]
                pyt = py.tile([128, 512], F32, tag="py")
                h2_sb = io.tile([16, 512], F32, tag="h2in")
                nc.sync.dma_start(h2_sb[:, :ow], h2[s, :, os_ : os_ + ow])

                # --- stage 2: 1x1 conv W1 (x2 = relu(W1@x1 + b1))
                x2_sb = work.tile([128, 4, 512], W_DT, tag="x2")
                for ot in range(4):
                    ps = px2.tile([128, 512], F32, tag="px2")
                    for kt in range(4):
                        nc.tensor.matmul(
                            ps[:, :ww],
                            w1_sb[:, kt, 128 * ot : 128 * (ot + 1)],
                            x1_cur[:, kt, :ww],
                            start=(kt == 0),
                            stop=(kt == 3),
                        )
                    nc.scalar.activation(
                        out=x2_sb[:, ot, :ww],
                        in_=ps[:, :ww],
                        func=AF.Relu,
                        bias=b1_sb[:, ot : ot + 1],
                        scale=1.0,
                    )

                # prefetch next window's stage 1 while ACT drains x2 relus
                if i + 1 < len(jobs):
                    s2, w2 = jobs[i + 1]
                    if w2 == 0:
                        stk_sb, dw_sb = nxt_dmas
                        cur_dmas = nxt_dmas
                        if s2 + 1 < S:
                            nxt_dmas = sample_dmas(s2 + 1)
                    x1_cur = stage1(s2, stk_sb, dw_sb, w2)

                # --- stage 3: 3-tap conv W2 as M=96 matmul (taps in M)
                for kt in range(4):
                    nc.tensor.matmul(
                        pyt[:96, :ww],
                        w2_sb[:, kt, :],
                        x2_sb[:, kt, :ww],
                        start=(kt == 0),
                        stop=(kt == 3),
                    )
                y_sb = work.tile([96, 512], W_DT, tag="y")
                nc.scalar.copy(out=y_sb[:, :ww], in_=pyt[:96, :ww])
                # reference zero-pads x2 outside [0, L): mask tap cols that
                # read the out-of-range x2 values
                if ws == -1:
                    nc.vector.memset(y_sb[0:32, 0:1], 0.0)
                # realign tap blocks to partitions 0:32 (engines cannot
                # cross partitions; DMA can)
                ya_sb = work.tile([32, 512], W_DT, tag="ya")
                yb_sb = work.tile([32, 512], W_DT, tag="yb")
                nc.sync.dma_start(ya_sb[:, :ww], y_sb[32:64, :ww])
                nc.sync.dma_start(yb_sb[:, :ww], y_sb[64:96, :ww])
                if ws + ww > L:
                    nc.vector.memset(yb_sb[:, L - ws : L - ws + 1], 0.0)
                # x3[j, n] = y0[j, n] + y1[j, n+1] + y2[j, n+2]
                x3_sb = work.tile([32, 512], W_DT, tag="x3")
                nc.vector.tensor_add(
                    out=x3_sb[:, :ow],
                    in0=y_sb[0:32, 0:ow],
                    in1=ya_sb[:, 1 : 1 + ow],
                )
                nc.vector.tensor_add(
                    out=x3_sb[:, :ow],
                    in0=x3_sb[:, :ow],
                    in1=yb_sb[:, 2 : 2 + ow],
                )
                # m half joins h2/s on partitions 0:16 -> one more remap
                x3m_sb = work.tile([16, 512], W_DT, tag="x3m")
                nc.sync.dma_start(x3m_sb[:, :ow], x3_sb[16:32, :ow])

                # s_ = sigmoid(x3_s + (b2[:16]+2))  (skip +1e-7: err <=1e-7)
                nc.scalar.activation(
                    out=s_all[:, s, os_ : os_ + ow],
                    in_=x3_sb[0:16, :ow],
                    func=AF.Sigmoid,
                    bias=b2s_sb,
                    scale=1.0,
                )
                # u = (x3_m + b2m) + h2 ; h2' = u * s_
                u_sb = io.tile([16, 512], F32, tag="u")
                nc.vector.scalar_tensor_tensor(
                    out=u_sb[:, :ow],
                    in0=x3m_sb[:, :ow],
                    scalar=b2m_sb,
                    in1=h2_sb[:, :ow],
                    op0=OP.add,
                    op1=OP.add,
                )
                ho_sb = io.tile([16, 512], F32, tag="ho")
                nc.vector.tensor_mul(
                    out=ho_sb[:, :ow],
                    in0=u_sb[:, :ow],
                    in1=s_all[:, s, os_ : os_ + ow],
                )
                nc.sync.dma_start(h2o[s, :, os_ : os_ + ow], ho_sb[:, :ow])

            # ---- phase 2: logdet = sum log s_  (one Ln table load) ----
            for s in range(S):
                lns = work.tile([16, L], F32, tag="lns")
                nc.scalar.activation(
                    out=lns[:],
                    in_=s_all[:, s, :],
                    func=AF.Ln,
                    accum_out=ld_all[:, s : s + 1],
                )
            pld = py.tile([128, 512], F32, tag="py")
            nc.tensor.matmul(
                pld[0:1, 0:S], ones_sb[:], ld_all[:], start=True, stop=True
            )
            ldo = io.tile([1, S], F32, tag="ldo")
            nc.vector.tensor_copy(ldo[:], pld[0:1, 0:S])
            nc.sync.dma_start(ld[:], ldo[:])

    nc.compile()
    return nc


_NC = None


def _get_nc():
    global _NC
    if _NC is None:
        _NC = build_bass()
    return _NC


def _pack_host(h, emb, Wa, ba, Wb, bb, W1, b1, W2, b2):
    f = np.float32
    emb = np.asarray(emb, f)
    w = (emb @ np.asarray(Wa, f).T + np.asarray(ba, f)).reshape(64, 512, KW)
    bvec = emb @ np.asarray(Wb, f).T + np.asarray(bb, f)  # [64, 512]
    wdt = ml_dtypes.bfloat16 if USE_BF16 else f
    wr = w.reshape(64, 4, 4, 32, KW)  # s, t, q, r, k
    # K=48 stacked layout: row 16k + 4t + q selects h1 chan 4t+q, tap k
    dwp_f = np.zeros((64, 48, 4, 128), f)
    for k in range(KW):
        for t in range(4):
            for q in range(4):
                dwp_f[:, 16 * k + 4 * t + q, t, 32 * q : 32 * q + 32] = wr[:, t, q, :, k]
    dwp = dwp_f.astype(wdt)
    bsb = np.ascontiguousarray(bvec.reshape(64, 4, 128).transpose(2, 0, 1))
    w1t = np.ascontiguousarray(np.asarray(W1, f)[:, :, 0].T.astype(wdt))
    w2t = np.ascontiguousarray(
        np.asarray(W2, f).transpose(1, 2, 0).reshape(512, 96).astype(wdt)
    )
    b1p = np.ascontiguousarray(np.asarray(b1, f).reshape(4, 128).T)
    b2 = np.asarray(b2, f)
    b2sp = np.ascontiguousarray((b2[:16] + 2.0).reshape(16, 1))
    b2mp = np.ascontiguousarray(b2[16:].reshape(16, 1))
    h = np.asarray(h, f)
    h1f = np.ascontiguousarray(h[:, :16, :])
    h2f = np.ascontiguousarray(h[:, 16:, :])
    h1pad = np.zeros((64, 16, PAD + L + 2), wdt)
    h1pad[:, :, PAD : PAD + L] = h1f.astype(wdt)
    hstk = np.zeros((64, 48, L + 2), wdt)
    for k in range(KW):
        hstk[:, 16 * k : 16 * (k + 1), :] = h1pad[:, :, k : k + L + 2]
    in_maps = []
    for c in range(NCORES):
        sl = slice(S * c, S * (c + 1))
        in_maps.append(
            {
                "h1": np.ascontiguousarray(hstk[sl]),
                "h2": np.ascontiguousarray(h2f[sl]),
                "dw": np.ascontiguousarray(dwp[sl]),
                "bsb": np.ascontiguousarray(bsb[:, sl, :]),
                "w1t": w1t,
                "w2t": w2t,
                "b1p": b1p,
                "b2s": b2sp,
                "b2m": b2mp,
            }
        )
    return h1f, in_maps


def run_on_device(in_maps, **kwargs):
    from concourse import bass_utils

    return bass_utils.run_bass_kernel_spmd(
        _get_nc(), in_maps, core_ids=list(range(NCORES)), **kwargs
    )


def kernel(h, emb, Wa, ba, Wb, bb, W1, b1, W2, b2):
    h1f, in_maps = _pack_host(h, emb, Wa, ba, Wb, bb, W1, b1, W2, b2)
    res = run_on_device(in_maps)
    outs = res.results
    h2o = np.concatenate([o["h2o"] for o in outs], axis=0)
    ldv = np.concatenate([o["ld"][0] for o in outs], axis=0)
    h_out = np.concatenate([h1f, h2o], axis=1)
    return h_out, ldv


# revision 20
# speedup vs baseline: 1.1968x; 1.1904x over previous
"""AffineCoupling Trainium2 kernel (8 NeuronCores, data-parallel over batch).

Reference computation (per sample s):
  w = emb@Wa.T+ba -> [512,3] dyn depthwise weights; b = emb@Wb.T+bb -> [512]
  x1 = relu(depthwise3(h1_bcast, w) + b)        [512, 2048]
  x2 = relu(W1 @ x1 + b1)                       [512, 2048]
  x3 = conv3(x2, W2) + b2                       [32, 2048]
  s_ = sigmoid(x3[:16]+2)+1e-7 ; m = x3[16:]
  h2' = s_*(h2+m); logdet = sum log s_
Host computes w,b (25 MFLOP) and packs weights; device does everything else.

Device mapping per core (8 samples):
 - L=2048 split into 5 column windows (4x 512-wide compute -> 510 out, +8).
 - depthwise: one K=48 matmul per 128-channel tile; rhs = 3 tap-shifted
   copies of the 16 h1 rows stacked on partitions (3 DMAs/sample), lhsT
   zero-padded so each tile selects its 4 h1 channels per tap.
 - W1 1x1 conv: 16 [128x128x512] matmuls/window, f32r full-rate fp32.
 - W2 3-tap conv: taps in M (M=96), 4 matmuls/window; tap-shift folded by
   DVE adds after partition realignment via SBUF-SBUF DMA.
 - sigmoid per window; Ln batched at the end (one ACT table load each).
"""

import os
import sys

sys.path.insert(0, "/opt/trn_rl_repo")

import ml_dtypes
import numpy as np

import concourse.mybir as mybir
import concourse.tile as tile
from concourse import bacc

KW = 3
NCORES = 8
S = 8          # samples per core
NSQC = 16      # h1 channels
L = 2048
PAD = 2        # left zero pad cols in padded h1 row buffer
F32 = mybir.dt.float32
MM_DT = mybir.dt.float32r  # fp32 data, full-rate PE path
USE_BF16 = os.environ.get("AC_BF16", "1") == "1"
# W1/W2 stage dtype: bf16 halves LDWEIGHTS (FWL) and keeps PE warm; the
# depthwise stays f32r so raw h1 is never rounded.
W_DT = mybir.dt.bfloat16 if USE_BF16 else MM_DT

# (win_start, win_w, out_start, out_w): compute window = out window +1 halo col
# each side; psum bank limit 512 -> 510 net output cols per window.
WINDOWS = [(2039, 10, 2040, 8)] + [(510 * i - 1, 512, 510 * i, 510) for i in range(4)]


def build_bass():
    nc = bacc.Bacc()
    AF = mybir.ActivationFunctionType
    OP = mybir.AluOpType

    h1 = nc.dram_tensor("h1", [S, 48, L + 2], W_DT, kind="ExternalInput")
    h2 = nc.dram_tensor("h2", [S, NSQC, L], F32, kind="ExternalInput")
    dw = nc.dram_tensor("dw", [S, 48, 4, 128], W_DT, kind="ExternalInput")
    bsb = nc.dram_tensor("bsb", [128, S, 4], F32, kind="ExternalInput")
    w1t = nc.dram_tensor("w1t", [512, 512], W_DT, kind="ExternalInput")
    w2t = nc.dram_tensor("w2t", [512, 96], W_DT, kind="ExternalInput")
    b1p = nc.dram_tensor("b1p", [128, 4], F32, kind="ExternalInput")
    b2s = nc.dram_tensor("b2s", [16, 1], F32, kind="ExternalInput")
    b2m = nc.dram_tensor("b2m", [16, 1], F32, kind="ExternalInput")
    h2o = nc.dram_tensor("h2o", [S, NSQC, L], F32, kind="ExternalOutput")
    ld = nc.dram_tensor("ld", [128, 1], F32, kind="ExternalOutput")

    with tile.TileContext(nc) as tc:
        with (
            tc.tile_pool(name="consts", bufs=1) as consts,
            tc.tile_pool(name="persample", bufs=2) as persample,
            tc.tile_pool(name="work", bufs=2) as work,
            tc.tile_pool(name="io", bufs=3) as io,
            tc.tile_pool(name="pdw", bufs=1, space="PSUM") as pdw,
            tc.tile_pool(name="px2", bufs=3, space="PSUM") as px2,
            tc.tile_pool(name="py", bufs=1, space="PSUM") as py,
        ):
            def sample_dmas(s):
                stk_sb = persample.tile(
                    [48, L + 2], W_DT, tag="stk", name="stk"
                )
                for k in range(KW):
                    nc.sync.dma_start(
                        stk_sb[16 * k : 16 * (k + 1), :], h1[s, 16 * k : 16 * (k + 1), :]
                    )
                dw_sb = persample.tile([48, 4, 128], W_DT, tag="dw", name="dwsb")
                nc.sync.dma_start(dw_sb[:], dw[s])
                return stk_sb, dw_sb

            def stage1(s, stk_sb, dw_sb, widx):
                ws, ww, os_, ow = WINDOWS[widx]
                pts = [
                    pdw.tile([128, 512], F32, tag=f"dw{t}", name=f"pdw{t}")
                    for t in range(4)
                ]
                for t in range(4):
                    nc.tensor.matmul(
                        pts[t][:, :ww],
                        dw_sb[:, t, :],
                        stk_sb[:, ws + 1 : ws + 1 + ww],
                        start=True,
                        stop=True,
                    )
                x1_sb = work.tile([128, 4, 512], W_DT, tag="x1", name="x1sb")
                for t in range(4):
                    if t < 2:
                        nc.vector.tensor_scalar(
                            out=x1_sb[:, t, :ww],
                            in0=pts[t][:, :ww],
                            scalar1=b_sb[:, s, t : t + 1],
                            scalar2=0.0,
                            op0=OP.add,
                            op1=OP.max,
                        )
                    else:
                        nc.scalar.activation(
                            out=x1_sb[:, t, :ww],
                            in_=pts[t][:, :ww],
                            func=AF.Relu,
                            bias=b_sb[:, s, t : t + 1],
                            scale=1.0,
                        )
                return x1_sb

            jobs = [(s, w) for s in range(S) for w in range(len(WINDOWS))]
            cur_dmas = sample_dmas(0)
            nxt_dmas = sample_dmas(1)
            # ---- constants ----
            w1_sb = consts.tile([128, 4, 512], W_DT)
            for kt in range(4):
                nc.sync.dma_start(w1_sb[:, kt, :], w1t[128 * kt : 128 * (kt + 1), :])
            w2_sb = consts.tile([128, 4, 96], W_DT)
            for kt in range(4):
                nc.sync.dma_start(w2_sb[:, kt, :], w2t[128 * kt : 128 * (kt + 1), :])
            b_sb = consts.tile([128, S, 4], F32)
            nc.sync.dma_start(b_sb[:], bsb[:])
            b1_sb = consts.tile([128, 4], F32)
            nc.sync.dma_start(b1_sb[:], b1p[:])
            b2s_sb = consts.tile([16, 1], F32)
            nc.sync.dma_start(b2s_sb[:], b2s[:])
            b2m_sb = consts.tile([16, 1], F32)
            nc.sync.dma_start(b2m_sb[:], b2m[:])
            s2_all = consts.tile([128, L], W_DT)
            ld_all = consts.tile([128, 1], F32)

            stk_sb, dw_sb = cur_dmas
            x1_cur = stage1(0, stk_sb, dw_sb, 0)
            for i, (s, widx) in enumerate(jobs):
                ws, ww, os_, ow = WINDOWS[widx]
                pyt = py.tile([128, 512], F32, tag="py")
                h2_sb = io.tile([16, 512], F32, tag="h2in")
                nc.gpsimd.dma_start(h2_sb[:, :ow], h2[s, :, os_ : os_ + ow])

                # --- stage 2: 1x1 conv W1 (x2 = relu(W1@x1 + b1))
                x2_sb = work.tile([128, 4, 512], W_DT, tag="x2")
                for ot in range(4):
                    ps = px2.tile([128, 512], F32, tag="px2")
                    for kt in range(4):
                        nc.tensor.matmul(
                            ps[:, :ww],
                            w1_sb[:, kt, 128 * ot : 128 * (ot + 1)],
                            x1_cur[:, kt, :ww],
                            start=(kt == 0),
                            stop=(kt == 3),
                        )
                    nc.scalar.activation(
                        out=x2_sb[:, ot, :ww],
                        in_=ps[:, :ww],
                        func=AF.Relu,
                        bias=b1_sb[:, ot : ot + 1],
                        scale=1.0,
                    )

                # prefetch next window's stage 1 while ACT drains x2 relus
                if i + 1 < len(jobs):
                    s2, w2 = jobs[i + 1]
                    if w2 == 0:
                        stk_sb, dw_sb = nxt_dmas
                        cur_dmas = nxt_dmas
                        if s2 + 1 < S:
                            nxt_dmas = sample_dmas(s2 + 1)
                    x1_cur = stage1(s2, stk_sb, dw_sb, w2)

                # --- stage 3: 3-tap conv W2 as M=96 matmul (taps in M)
                for kt in range(4):
                    nc.tensor.matmul(
                        pyt[:96, :ww],
                        w2_sb[:, kt, :],
                        x2_sb[:, kt, :ww],
                        start=(kt == 0),
                        stop=(kt == 3),
                    )
                y_sb = work.tile([96, 512], W_DT, tag="y")
                nc.scalar.copy(out=y_sb[:, :ww], in_=pyt[:96, :ww])
                # reference zero-pads x2 outside [0, L): mask tap cols that
                # read the out-of-range x2 values
                if ws == -1:
                    nc.vector.memset(y_sb[0:32, 0:1], 0.0)
                # realign tap blocks to partitions 0:32 (engines cannot
                # cross partitions; DMA can)
                ya_sb = work.tile([32, 512], W_DT, tag="ya")
                yb_sb = work.tile([32, 512], W_DT, tag="yb")
                nc.sync.dma_start(ya_sb[:, :ww], y_sb[32:64, :ww])
                nc.gpsimd.dma_start(yb_sb[:, :ww], y_sb[64:96, :ww])
                if ws + ww > L:
                    nc.vector.memset(yb_sb[:, L - ws : L - ws + 1], 0.0)
                # x3[j, n] = y0[j, n] + y1[j, n+1] + y2[j, n+2]
                x3_sb = work.tile([32, 512], W_DT, tag="x3")
                nc.vector.tensor_add(
                    out=x3_sb[:, :ow],
                    in0=y_sb[0:32, 0:ow],
                    in1=ya_sb[:, 1 : 1 + ow],
                )
                nc.vector.tensor_add(
                    out=x3_sb[:, :ow],
                    in0=x3_sb[:, :ow],
                    in1=yb_sb[:, 2 : 2 + ow],
                )
                # m half joins h2/s on partitions 0:16 -> one more remap
                x3m_sb = work.tile([16, 512], W_DT, tag="x3m")
                nc.sync.dma_start(x3m_sb[:, :ow], x3_sb[16:32, :ow])

                # s_ = sigmoid(x3_s + (b2[:16]+2))  (skip +1e-7: err <=1e-7)
                s_sb = io.tile([16, 512], W_DT, tag="ssb")
                nc.scalar.activation(
                    out=s_sb[:, :ow],
                    in_=x3_sb[0:16, :ow],
                    func=AF.Sigmoid,
                    bias=b2s_sb,
                    scale=1.0,
                )
                nc.gpsimd.dma_start(
                    s2_all[16 * s : 16 * (s + 1), os_ : os_ + ow], s_sb[:, :ow]
                )
                # u = (x3_m + b2m) + h2 ; h2' = u * s_
                u_sb = io.tile([16, 512], F32, tag="u")
                nc.vector.scalar_tensor_tensor(
                    out=u_sb[:, :ow],
                    in0=x3m_sb[:, :ow],
                    scalar=b2m_sb,
                    in1=h2_sb[:, :ow],
                    op0=OP.add,
                    op1=OP.add,
                )
                ho_sb = io.tile([16, 512], F32, tag="ho")
                nc.vector.tensor_mul(
                    out=ho_sb[:, :ow],
                    in0=u_sb[:, :ow],
                    in1=s_sb[:, :ow],
                )
                nc.gpsimd.dma_start(h2o[s, :, os_ : os_ + ow], ho_sb[:, :ow])

            # ---- phase 2: logdet partials = rowsum log s_ (single Ln op) ----
            lns = work.tile([128, L], F32, tag="lns")
            nc.scalar.activation(
                out=lns[:],
                in_=s2_all[:],
                func=AF.Ln,
                accum_out=ld_all[:],
            )
            nc.sync.dma_start(ld[:], ld_all[:])

    nc.compile()
    return nc


_NC = None


def _get_nc():
    global _NC
    if _NC is None:
        _NC = build_bass()
    return _NC


def _pack_host(h, emb, Wa, ba, Wb, bb, W1, b1, W2, b2):
    f = np.float32
    emb = np.asarray(emb, f)
    w = (emb @ np.asarray(Wa, f).T + np.asarray(ba, f)).reshape(64, 512, KW)
    bvec = emb @ np.asarray(Wb, f).T + np.asarray(bb, f)  # [64, 512]
    wdt = ml_dtypes.bfloat16 if USE_BF16 else f
    wr = w.reshape(64, 4, 4, 32, KW)  # s, t, q, r, k
    # K=48 stacked layout: row 16k + 4t + q selects h1 chan 4t+q, tap k
    dwp_f = np.zeros((64, 48, 4, 128), f)
    for k in range(KW):
        for t in range(4):
            for q in range(4):
                dwp_f[:, 16 * k + 4 * t + q, t, 32 * q : 32 * q + 32] = wr[:, t, q, :, k]
    dwp = dwp_f.astype(wdt)
    bsb = np.ascontiguousarray(bvec.reshape(64, 4, 128).transpose(2, 0, 1))
    w1t = np.ascontiguousarray(np.asarray(W1, f)[:, :, 0].T.astype(wdt))
    w2t = np.ascontiguousarray(
        np.asarray(W2, f).transpose(1, 2, 0).reshape(512, 96).astype(wdt)
    )
    b1p = np.ascontiguousarray(np.asarray(b1, f).reshape(4, 128).T)
    b2 = np.asarray(b2, f)
    b2sp = np.ascontiguousarray((b2[:16] + 2.0).reshape(16, 1))
    b2mp = np.ascontiguousarray(b2[16:].reshape(16, 1))
    h = np.asarray(h, f)
    h1f = np.ascontiguousarray(h[:, :16, :])
    h2f = np.ascontiguousarray(h[:, 16:, :])
    h1pad = np.zeros((64, 16, PAD + L + 2), wdt)
    h1pad[:, :, PAD : PAD + L] = h1f.astype(wdt)
    hstk = np.zeros((64, 48, L + 2), wdt)
    for k in range(KW):
        hstk[:, 16 * k : 16 * (k + 1), :] = h1pad[:, :, k : k + L + 2]
    in_maps = []
    for c in range(NCORES):
        sl = slice(S * c, S * (c + 1))
        in_maps.append(
            {
                "h1": np.ascontiguousarray(hstk[sl]),
                "h2": np.ascontiguousarray(h2f[sl]),
                "dw": np.ascontiguousarray(dwp[sl]),
                "bsb": np.ascontiguousarray(bsb[:, sl, :]),
                "w1t": w1t,
                "w2t": w2t,
                "b1p": b1p,
                "b2s": b2sp,
                "b2m": b2mp,
            }
        )
    return h1f, in_maps


def run_on_device(in_maps, **kwargs):
    from concourse import bass_utils

    return bass_utils.run_bass_kernel_spmd(
        _get_nc(), in_maps, core_ids=list(range(NCORES)), **kwargs
    )


def kernel(h, emb, Wa, ba, Wb, bb, W1, b1, W2, b2):
    h1f, in_maps = _pack_host(h, emb, Wa, ba, Wb, bb, W1, b1, W2, b2)
    res = run_on_device(in_maps)
    outs = res.results
    h2o = np.concatenate([o["h2o"] for o in outs], axis=0)
    ldv = np.concatenate(
        [o["ld"].reshape(S, 16).sum(axis=1) for o in outs], axis=0
    )
    h_out = np.concatenate([h1f, h2o], axis=1)
    return h_out, ldv
